# revision 9
# baseline (speedup 1.0000x reference)
"""GNN message passing (gather + weighted segment-sum) on 8 Trainium2 cores.

out[n, :] = sum_{e : dst[e] == n} weight[e] * queue[src[e], :]

Design
------
Edges are sharded by destination window (128 dst nodes per window, 49
windows per core), bucketed per (window, src-parity) and sorted by src.
The queue is stored in HBM as fp16 node-PAIRS: qp[t] = [fp16(queue[2t]) |
fp16(queue[2t+1])] (256B rows — the dma_gather element-size floor), so an
edge's row index is src>>1 (fits int16) and the parity picks the useful
64-value half.  Each core per iteration:
  * window-level dma_gathers of qp rows per (chunk, window, parity) —
    pads point at row 0 and carry weight 0,
  * one DVE pass per (chunk, parity) folds the edge weight into the
    gathered rows: gw = g[:, parity-half] * w  (fp16),
  * one DVE is_equal pass per window builds the pure one-hot H01 from
    iota vs per-edge dst offsets,
  * ONE matmul per 128-edge block: ps[128 dst, 64] += H01_blk^T @ gw_blk
    (vs 3 matmul terms in the hi/lo-bf16 formulation),
  * ACT copies ps -> SBUF, DMA to the output slice.

PSUM accumulator tiles are padded to full 2KB banks: a start=True matmul
clears the whole bank's has_written bits, so accumulators must not share
banks with other matmul groups.

All data-dependent structure is computed on the host from the actual
inputs at call time; all 8 cores run one SPMD program.
"""

import contextlib
import sys

sys.path.insert(0, "/opt/trn_rl_repo")

import ml_dtypes  # noqa: F401
import numpy as np

import concourse.bass as bass  # noqa: F401
import concourse.mybir as mybir
import concourse.tile as tile
from concourse import bacc
from concourse.bass_utils import run_bass_kernel_spmd

P = 128
N_CORES = 8

N_NODES = 50000
N_EDGES = 800000
D_FEAT = 64

NPAIR = N_NODES // 2  # fp16 pair-rows in qp
RANKS = -(-NPAIR // P)  # stripe columns in the SBUF-resident queue
NPAD = RANKS * P  # qp rows incl. padding (rank grid)


def _plan(n_nodes, n_cores):
    n_windows = -(-n_nodes // P)
    wpc = -(-n_windows // n_cores)
    cw = max(d for d in range(1, min(wpc, 8) + 1) if wpc % d == 0)
    nchunk = wpc // cw
    return wpc, cw, nchunk


def _host_prep(weight, src, dst, n_nodes, wpc, cw, nchunk, n_cores):
    """Bucket edges by (core, window, src parity); pad uniformly.

    Returns (epw, idx_hbm, aux_hbm):
      idx_hbm [n_cores, nchunk, 2, 128, cw*epw//16] int16 (dma_gather layout,
              pads -> row 0)
      aux_hbm [n_cores, nchunk, 128, 4*cw*nb] fp16:
              cols [0, 2*cw*nb)         dstoff, window-major (j*2+h)*nb+k
              cols [2*cw*nb, 4*cw*nb)   weight, half-major   h*cw*nb+j*nb+k
    """
    e = src.shape[0]
    src = np.asarray(src).astype(np.int64).reshape(-1)
    dst = np.asarray(dst).astype(np.int64).reshape(-1)
    wgt = np.asarray(weight, dtype=np.float32).reshape(-1)

    w = dst >> 7
    core = w // wpc
    lw = w - core * wpc
    half = src & 1
    hidx = (src >> 1).astype(np.int16)
    dstoff = (dst & 127).astype(np.float32)

    nbuckets = n_cores * wpc * 2
    key = (core * wpc + lw) * 2 + half
    order = np.lexsort((src, key))
    counts = np.bincount(key, minlength=nbuckets)
    epw = int(-(-max(int(counts.max()), 1) // P) * P)
    offs = np.zeros(nbuckets + 1, np.int64)
    np.cumsum(counts, out=offs[1:])
    skey = key[order]
    rank = np.arange(e, dtype=np.int64) - offs[skey]
    dest = skey * epw + rank

    f16 = np.float16
    idx_arr = np.zeros(nbuckets * epw, np.int16)  # pads gather row 0
    dst_arr = np.zeros(nbuckets * epw, f16)
    wgt_arr = np.zeros(nbuckets * epw, f16)  # pads carry weight 0
    idx_arr[dest] = hidx[order]
    dst_arr[dest] = dstoff[order].astype(f16)  # 0..127, exact
    wgt_arr[dest] = wgt[order].astype(f16)

    nb = epw // P
    big = cw * epw
    shp = (n_cores, nchunk, cw, 2, epw)
    idx_arr = idx_arr.reshape(shp)
    dst_arr = dst_arr.reshape(shp)
    wgt_arr = wgt_arr.reshape(shp)

    # idx: half-major edge list per (core, chunk), window-major within a half,
    # wrapped mod 16 and replicated to 128 partitions.
    a = idx_arr.transpose(0, 1, 3, 2, 4).reshape(n_cores, nchunk, 2, big // 16, 16)
    a = a.transpose(0, 1, 2, 4, 3)
    idx_hbm = np.broadcast_to(
        a[:, :, :, None, :, :], (n_cores, nchunk, 2, 8, 16, big // 16)
    ).reshape(n_cores, nchunk, 2, P, big // 16)
    idx_hbm = np.ascontiguousarray(idx_hbm)

    def pack_wmaj(x):
        # window-major block columns: col = (j*2 + h)*nb + k
        y = x.reshape(n_cores, nchunk, cw, 2, nb, P)
        y = y.transpose(0, 1, 5, 2, 3, 4)  # [core, chunk, P, j, h, k]
        return y.reshape(n_cores, nchunk, P, 2 * cw * nb)

    def pack_hmaj(x):
        # half-major block columns: col = h*cw*nb + j*nb + k
        y = x.reshape(n_cores, nchunk, cw, 2, nb, P)
        y = y.transpose(0, 1, 5, 3, 2, 4)  # [core, chunk, P, h, j, k]
        return y.reshape(n_cores, nchunk, P, 2 * cw * nb)

    aux_hbm = np.concatenate([pack_wmaj(dst_arr), pack_hmaj(wgt_arr)], axis=3)
    aux_hbm = np.ascontiguousarray(aux_hbm)
    wgt32 = np.zeros(nbuckets * epw, np.float32)
    wgt32[dest] = wgt[order]
    wgt32 = wgt32.reshape(shp)
    y = wgt32.reshape(n_cores, nchunk, cw, 2, nb, P)
    y = y.transpose(0, 1, 5, 3, 2, 4)  # half-major, matches aux weight cols
    wgt_hbm = np.ascontiguousarray(y.reshape(n_cores, nchunk, P, 2 * cw * nb))
    cnt_hbm = np.ascontiguousarray(
        counts.reshape(n_cores, nchunk, cw, 2)
        .transpose(0, 1, 3, 2)  # [core, chunk, half, window]
        .reshape(n_cores, 1, nchunk * 2 * cw)
        .astype(np.int32)
    )
    return epw, idx_hbm, aux_hbm, wgt_hbm, cnt_hbm


ALL_PARTS = frozenset({"gather", "dve", "mm", "out"})

KSB = (0, 0)  # SBUF-path window count; 0 = all edges via HBM gather
# (the SBUF-resident transpose-gather path computes correctly in isolation
# but its RX completion is under-synchronized in integration; disabled)


def _build(
    n_nodes, d, epw, wpc, cw, nchunk, iters=1, parts=ALL_PARTS, nq=4, ksb=KSB,
    trim=False,
):
    f32 = mybir.dt.float32
    f16 = mybir.dt.float16
    nb = epw // P
    big = cw * epw
    cwnb = cw * nb
    assert n_nodes % 2 == 0

    nc = bacc.Bacc(
        "TRN2", target_bir_lowering=False, debug=False, num_swdge_queues=nq
    )

    qp_t = nc.dram_tensor("qp", [NPAD, 2 * d], f16, kind="ExternalInput")
    qps_t = nc.dram_tensor("qps", [P, RANKS * 2 * d], f16, kind="ExternalInput")
    idx_t = nc.dram_tensor(
        "idx", [nchunk, 2, P, big // 16], mybir.dt.int16, kind="ExternalInput"
    )
    aux_t = nc.dram_tensor("aux", [nchunk, P, 4 * cwnb], f16, kind="ExternalInput")
    wgt_t = nc.dram_tensor("wgt", [nchunk, P, 2 * cwnb], f32, kind="ExternalInput")
    cnt_t = nc.dram_tensor(
        "cnt", [1, nchunk * 2 * cw], mybir.dt.int32, kind="ExternalInput"
    )
    iota_t = nc.dram_tensor("iota", [P, P], f16, kind="ExternalInput")
    ident_t = nc.dram_tensor("ident", [P, P], f16, kind="ExternalInput")
    out_t = nc.dram_tensor("out", [wpc * P, d], f32, kind="ExternalOutput")

    qview = qp_t.ap()[:, 0 : 2 * d]

    def bcast(ap2d, n_mid, mid_is_data):
        pairs = list(ap2d.ap)
        assert len(pairs) == 2
        if mid_is_data:
            newp = [pairs[0], [pairs[1][0], n_mid], [0, P]]
        else:
            newp = [pairs[0], [0, n_mid], pairs[1]]
        return bass.AP(ap2d.tensor, ap2d.offset, newp)

    def bcast64(ap2d, n_mid):
        pairs = list(ap2d.ap)
        assert len(pairs) == 2
        newp = [pairs[0], [pairs[1][0], n_mid], [0, 64]]
        return bass.AP(ap2d.tensor, ap2d.offset, newp)

    with tile.TileContext(nc) as tc:
        with (
            tc.tile_pool(name="const", bufs=1) as cpool,
            tc.tile_pool(name="io", bufs=3) as iopool,
            tc.tile_pool(name="gat", bufs=4) as gpool,
            tc.tile_pool(name="gts", bufs=2) as tpool,
            tc.tile_pool(name="gw", bufs=2) as wpool,
            tc.tile_pool(name="gwsb", bufs=10) as spool,
            tc.tile_pool(name="hot", bufs=3) as hpool,
            tc.tile_pool(name="ost", bufs=4) as opool,
            tc.tile_pool(name="ps", bufs=4, space="PSUM") as ppool,
            tc.tile_pool(name="pst", bufs=4, space="PSUM") as ppool2,
        ):
            iota_f = cpool.tile([P, P], f16)
            nc.sync.dma_start(out=iota_f[:], in_=iota_t.ap()[:, :])
            ident = cpool.tile([P, P], f16)
            nc.sync.dma_start(out=ident[:], in_=ident_t.ap()[:, :])
            qsb = None
            if max(ksb) > 0:
                qsb = cpool.tile([P, RANKS * 2 * d], f16)
            cnt = None
            if trim:
                cnt = cpool.tile([1, nchunk * 2 * cw], mybir.dt.int32)
                nc.sync.dma_start(out=cnt[:], in_=cnt_t.ap()[:, :])
            # pre-zero HBM gather slots: reg-trimmed tails are never written
            # and must not hold NaN bit patterns on the first iteration
            for h in (0, 1):
                if cw > ksb[h]:
                    for _ in range(4):
                        gz = gpool.tile(
                            [P, (cw - ksb[h]) * nb, 2 * d], f16, tag=f"g{h}"
                        )
                        nc.vector.memset(gz[:], 0)

            loop = tc.For_i(0, iters, 1) if iters > 1 else contextlib.nullcontext()
            with loop:
                # stage the queue into SBUF (stripe-major node-pair tokens)
                if "gather" in parts and qsb is not None:
                    nc.sync.dma_start(out=qsb[:], in_=qps_t.ap()[:, :])
                for c in range(nchunk):
                    idxs = []
                    for h in (0, 1):
                        it = iopool.tile(
                            [P, big // 16], mybir.dt.int16, tag=f"idx{h}"
                        )
                        nc.sync.dma_start(out=it[:], in_=idx_t.ap()[c, h])
                        idxs.append(it)
                    aux = iopool.tile([P, 4 * cwnb], f16, tag="aux")
                    nc.sync.dma_start(out=aux[:], in_=aux_t.ap()[c])
                    wf = None
                    if max(ksb) > 0:
                        wf = iopool.tile([P, 2 * cwnb], f32, tag="wgt")
                        nc.sync.dma_start(out=wf[:], in_=wgt_t.ap()[c])

                    sl = epw // 16
                    ghbm = []
                    gws = []
                    gsb = {}
                    # HBM-path gathers first: they are the long pole
                    for h in (0, 1):
                        k = ksb[h]
                        g = None
                        if cw > k:
                            g = gpool.tile(
                                [P, (cw - k) * nb, 2 * d], f16, tag=f"g{h}"
                            )
                        if "gather" in parts and cw > k:
                            nqh = 2 if max(ksb) > 0 else nq
                            for j in range(k, cw):
                                jj = j - k
                                if trim:
                                    r = nc.alloc_register(mybir.EngineType.Pool)
                                    gi = (c * 2 + h) * cw + j
                                    nc.gpsimd.reg_load(r, cnt[0:1, gi : gi + 1])
                                else:
                                    r = epw
                                nc.gpsimd.dma_gather(
                                    out_ap=g[:, jj * nb : (jj + 1) * nb, :],
                                    in_ap=qview,
                                    idxs_ap=idxs[h][:, j * sl : (j + 1) * sl],
                                    num_idxs=epw,
                                    num_idxs_reg=r,
                                    elem_size=2 * d,
                                    elem_step=2 * d,
                                    single_packet=False,
                                    queue_num=(j + h) % nqh,
                                )
                        ghbm.append(g)
                    # SBUF-path gathers: windows [0, k), window-level
                    for j in range(max(ksb)):
                        for h in (0, 1):
                            if j >= ksb[h]:
                                continue
                            gt = tpool.tile([P, 1, epw], f16, tag=f"gt{h}{j}")
                            if "gather" in parts:
                                nc.gpsimd.dma_gather(
                                    out_ap=gt[:],
                                    in_ap=qsb[:],
                                    idxs_ap=idxs[h][:, j * sl : (j + 1) * sl],
                                    num_idxs=epw,
                                    num_idxs_reg=epw,
                                    elem_size=2 * d,
                                    transpose=True,
                                    single_packet=False,
                                    queue_num=2 + ((j + h) % 2),
                                    sbuf_tokens_per_rank=P,
                                    sbuf_free_dim_per_rank=4 * d,
                                    sbuf_free_dim_pad_per_rank=0,
                                    sbuf_byte_offset=0,
                                )
                            gsb[(h, j)] = gt
                    for h in (0, 1):
                        k = ksb[h]
                        gw = None
                        if cw > k:
                            gw = wpool.tile([P, (cw - k) * nb, d], f16, tag=f"gw{h}")
                        if "dve" in parts and cw > k:
                            # gw = g[:, :, h*64:(h+1)*64] * w (per edge)
                            nc.vector.tensor_tensor(
                                out=gw[:],
                                in0=ghbm[h][:, :, h * d : (h + 1) * d],
                                in1=bcast64(
                                    aux[
                                        :,
                                        (2 + h) * cwnb + k * nb : (3 + h) * cwnb,
                                    ],
                                    (cw - k) * nb,
                                ),
                                op=mybir.AluOpType.mult,
                            )
                        gws.append(gw)

                    for j in range(cw):
                        nbw = 2 * nb
                        wcol = j * nbw
                        h01 = hpool.tile([P, nbw, P], f16, tag="h01")
                        if "dve" in parts:
                            nc.vector.tensor_tensor(
                                out=h01[:],
                                in0=bcast(iota_f[:], nbw, False),
                                in1=bcast(aux[:, wcol : wcol + nbw], nbw, True),
                                op=mybir.AluOpType.is_equal,
                            )
                        # SBUF-path rhs blocks: transpose + weight on ACT
                        rhs_sb = {}
                        for h in (0, 1):
                            if j >= ksb[h] or "gather" not in parts:
                                continue
                            gt = gsb[(h, j)]
                            for k2 in range(nb):
                                # transpose as a plain matmul: gt_blk.T @ I
                                # (full-bank PSUM tile: start=True clears the
                                # whole bank's has_written bits, so no other
                                # accumulator may share this bank)
                                pt = ppool2.tile([P, 512], f32, tag="pt")
                                nc.tensor.matmul(
                                    pt[:, 0:P],
                                    lhsT=gt[:, 0, k2 * P : (k2 + 1) * P],
                                    rhs=ident[:],
                                    start=True,
                                    stop=True,
                                )
                                gwb = spool.tile([P, 64], f16, tag=f"gwb{h}")
                                wc = h * cwnb + j * nb + k2
                                nc.scalar.mul(
                                    gwb[:],
                                    pt[:, h * 64 : (h + 1) * 64],
                                    wf[:, wc : wc + 1],
                                )
                                rhs_sb[(h, k2)] = gwb
                        ps = ppool.tile([P, 512], f32)
                        if "mm" in parts:
                            first = True
                            for h in (0, 1):
                                k = ksb[h]
                                for k2 in range(nb):
                                    bi = h * nb + k2
                                    lhs = (
                                        h01[:, bi, :]
                                        if "dve" in parts
                                        else iota_f[:]
                                    )
                                    if j < k and "gather" in parts:
                                        rhs = rhs_sb[(h, k2)][:]
                                    else:
                                        jj = max(j - k, 0)
                                        rhs = gws[h][:, jj * nb + k2, :]
                                    nc.tensor.matmul(
                                        ps[:, 0:d],
                                        lhsT=lhs,
                                        rhs=rhs,
                                        start=first,
                                        stop=(h == 1 and k2 == nb - 1),
                                    )
                                    first = False
                        wg = c * cw + j
                        if "out" in parts and "mm" in parts:
                            ot = opool.tile([P, d], f32, tag="ot")
                            nc.scalar.copy(ot[:], ps[:, 0:d])
                            nc.sync.dma_start(
                                out=out_t.ap()[wg * P : (wg + 1) * P, :], in_=ot[:]
                            )
                        elif "mm" not in parts:
                            if "dve" in parts:
                                srcs = [h01[:, 0, :]]
                            else:
                                srcs = [
                                    g[:, 0, 0:P] for g in ghbm if g is not None
                                ]
                            for src_ap in srcs:
                                nc.sync.dma_start(
                                    out=out_t.ap()[wg * P : (wg + 1) * P, 0:32],
                                    in_=src_ap.bitcast(f32)[:, 0:32],
                                )
    nc.compile()
    return nc


def _make_inputs(queue, idx_hbm, aux_hbm, wgt_hbm, cnt_hbm, n_cores):
    f16 = np.float16
    q = np.asarray(queue, dtype=np.float32)
    d = q.shape[1]
    qp = np.zeros((NPAD, 2 * d), f16)
    qp[:NPAIR, 0:d] = q[0::2].astype(f16)
    qp[:NPAIR, d : 2 * d] = q[1::2].astype(f16)
    # stripe-major staging image: token t -> partition t & 127,
    # free cols (t>>7)*2d .. +2d
    qps = np.ascontiguousarray(
        qp.reshape(RANKS, P, 2 * d).transpose(1, 0, 2).reshape(P, RANKS * 2 * d)
    )
    iota_np = np.ascontiguousarray(
        np.broadcast_to(np.arange(P, dtype=np.float32), (P, P)).astype(f16)
    )
    ident_np = np.ascontiguousarray(np.eye(P, dtype=f16))
    return [
        {
            "qp": qp,
            "qps": qps,
            "idx": idx_hbm[c],
            "aux": aux_hbm[c],
            "wgt": wgt_hbm[c],
            "cnt": cnt_hbm[c],
            "iota": iota_np,
            "ident": ident_np,
        }
        for c in range(n_cores)
    ]


def _run(queue, weight, src, dst, n_nodes, d, n_cores, iters=1):
    queue = np.ascontiguousarray(np.asarray(queue, dtype=np.float32))
    wpc, cw, nchunk = _plan(n_nodes, n_cores)
    epw, idx_hbm, aux_hbm, wgt_hbm, cnt_hbm = _host_prep(
        weight, src, dst, n_nodes, wpc, cw, nchunk, n_cores
    )
    nc = _build(n_nodes, d, epw, wpc, cw, nchunk, iters=iters)
    in_maps = _make_inputs(queue, idx_hbm, aux_hbm, wgt_hbm, cnt_hbm, n_cores)
    res = run_bass_kernel_spmd(nc, in_maps, core_ids=list(range(n_cores)))
    full = np.concatenate([res.results[c]["out"] for c in range(n_cores)], axis=0)
    return full[:n_nodes], res


def kernel(queue, weight, src, dst):
    out, _ = _run(queue, weight, src, dst, N_NODES, D_FEAT, N_CORES)
    return out


# revision 10
# speedup vs baseline: 1.1377x; 1.1377x over previous
"""GNN message passing (gather + weighted segment-sum) on 8 Trainium2 cores.

out[n, :] = sum_{e : dst[e] == n} weight[e] * queue[src[e], :]

Design
------
Edges are sharded by destination window (128 dst nodes per window, 49
windows per core), bucketed per (window, src-parity) and sorted by src.
The queue is stored in HBM as fp16 node-PAIRS: qp[t] = [fp16(queue[2t]) |
fp16(queue[2t+1])] (256B rows — the dma_gather element-size floor), so an
edge's row index is src>>1 (fits int16) and the parity picks the useful
64-value half.  Each core per iteration:
  * window-level dma_gathers of qp rows per (chunk, window, parity) —
    pads point at row 0 and carry weight 0,
  * one DVE pass per (chunk, parity) folds the edge weight into the
    gathered rows: gw = g[:, parity-half] * w  (fp16),
  * one DVE is_equal pass per window builds the pure one-hot H01 from
    iota vs per-edge dst offsets,
  * ONE matmul per 128-edge block: ps[128 dst, 64] += H01_blk^T @ gw_blk
    (vs 3 matmul terms in the hi/lo-bf16 formulation),
  * ACT copies ps -> SBUF, DMA to the output slice.

PSUM accumulator tiles are padded to full 2KB banks: a start=True matmul
clears the whole bank's has_written bits, so accumulators must not share
banks with other matmul groups.

All data-dependent structure is computed on the host from the actual
inputs at call time; all 8 cores run one SPMD program.
"""

import contextlib
import sys

sys.path.insert(0, "/opt/trn_rl_repo")

import ml_dtypes  # noqa: F401
import numpy as np

import concourse.bass as bass  # noqa: F401
import concourse.mybir as mybir
import concourse.tile as tile
from concourse import bacc
from concourse.bass_utils import run_bass_kernel_spmd

P = 128
N_CORES = 8

N_NODES = 50000
N_EDGES = 800000
D_FEAT = 64

NPAIR = N_NODES // 2  # fp16 pair-rows in qp
RANKS = -(-NPAIR // P)  # stripe columns in the SBUF-resident queue
NPAD = RANKS * P  # qp rows incl. padding (rank grid)


def _plan(n_nodes, n_cores):
    n_windows = -(-n_nodes // P)
    wpc = -(-n_windows // n_cores)
    cw = max(d for d in range(1, min(wpc, 8) + 1) if wpc % d == 0)
    nchunk = wpc // cw
    return wpc, cw, nchunk


def _host_prep(weight, src, dst, n_nodes, wpc, cw, nchunk, n_cores):
    """Bucket edges by (core, window, src parity); pad uniformly.

    Returns (epw, idx_hbm, aux_hbm):
      idx_hbm [n_cores, nchunk, 2, 128, cw*epw//16] int16 (dma_gather layout,
              pads -> row 0)
      aux_hbm [n_cores, nchunk, 128, 4*cw*nb] fp16:
              cols [0, 2*cw*nb)         dstoff, window-major (j*2+h)*nb+k
              cols [2*cw*nb, 4*cw*nb)   weight, half-major   h*cw*nb+j*nb+k
    """
    e = src.shape[0]
    src = np.asarray(src).astype(np.int64).reshape(-1)
    dst = np.asarray(dst).astype(np.int64).reshape(-1)
    wgt = np.asarray(weight, dtype=np.float32).reshape(-1)

    w = dst >> 7
    core = w // wpc
    lw = w - core * wpc
    half = src & 1
    hidx = (src >> 1).astype(np.int16)
    dstoff = (dst & 127).astype(np.float32)

    nbuckets = n_cores * wpc * 2
    key = (core * wpc + lw) * 2 + half
    order = np.lexsort((src, key))
    counts = np.bincount(key, minlength=nbuckets)
    epw = int(-(-max(int(counts.max()), 1) // P) * P)
    offs = np.zeros(nbuckets + 1, np.int64)
    np.cumsum(counts, out=offs[1:])
    skey = key[order]
    rank = np.arange(e, dtype=np.int64) - offs[skey]
    dest = skey * epw + rank

    f16 = np.float16
    idx_arr = np.zeros(nbuckets * epw, np.int16)  # pads gather row 0
    dst_arr = np.zeros(nbuckets * epw, f16)
    wgt_arr = np.zeros(nbuckets * epw, f16)  # pads carry weight 0
    idx_arr[dest] = hidx[order]
    dst_arr[dest] = dstoff[order].astype(f16)  # 0..127, exact
    wgt_arr[dest] = wgt[order].astype(f16)

    nb = epw // P
    big = cw * epw
    shp = (n_cores, nchunk, cw, 2, epw)
    idx_arr = idx_arr.reshape(shp)
    dst_arr = dst_arr.reshape(shp)
    wgt_arr = wgt_arr.reshape(shp)

    # idx: half-major edge list per (core, chunk), window-major within a half,
    # wrapped mod 16 and replicated to 128 partitions.
    a = idx_arr.transpose(0, 1, 3, 2, 4).reshape(n_cores, nchunk, 2, big // 16, 16)
    a = a.transpose(0, 1, 2, 4, 3)
    idx_hbm = np.broadcast_to(
        a[:, :, :, None, :, :], (n_cores, nchunk, 2, 8, 16, big // 16)
    ).reshape(n_cores, nchunk, 2, P, big // 16)
    idx_hbm = np.ascontiguousarray(idx_hbm)

    def pack_wmaj(x):
        # window-major block columns: col = (j*2 + h)*nb + k
        y = x.reshape(n_cores, nchunk, cw, 2, nb, P)
        y = y.transpose(0, 1, 5, 2, 3, 4)  # [core, chunk, P, j, h, k]
        return y.reshape(n_cores, nchunk, P, 2 * cw * nb)

    def pack_hmaj(x):
        # half-major block columns: col = h*cw*nb + j*nb + k
        y = x.reshape(n_cores, nchunk, cw, 2, nb, P)
        y = y.transpose(0, 1, 5, 3, 2, 4)  # [core, chunk, P, h, j, k]
        return y.reshape(n_cores, nchunk, P, 2 * cw * nb)

    aux_hbm = np.concatenate([pack_wmaj(dst_arr), pack_hmaj(wgt_arr)], axis=3)
    aux_hbm = np.ascontiguousarray(aux_hbm)
    wgt32 = np.zeros(nbuckets * epw, np.float32)
    wgt32[dest] = wgt[order]
    wgt32 = wgt32.reshape(shp)
    y = wgt32.reshape(n_cores, nchunk, cw, 2, nb, P)
    y = y.transpose(0, 1, 5, 3, 2, 4)  # half-major, matches aux weight cols
    wgt_hbm = np.ascontiguousarray(y.reshape(n_cores, nchunk, P, 2 * cw * nb))
    cnt_hbm = np.ascontiguousarray(
        counts.reshape(n_cores, nchunk, cw, 2)
        .transpose(0, 1, 3, 2)  # [core, chunk, half, window]
        .reshape(n_cores, 1, nchunk * 2 * cw)
        .astype(np.int32)
    )
    return epw, idx_hbm, aux_hbm, wgt_hbm, cnt_hbm


ALL_PARTS = frozenset({"gather", "dve", "mm", "out"})

KSB = (0, 0)  # SBUF-path window count; 0 = all edges via HBM gather
# (the SBUF-resident transpose-gather path computes correctly in isolation
# but its RX completion is under-synchronized in integration; disabled)


def _build(
    n_nodes, d, epw, wpc, cw, nchunk, iters=1, parts=ALL_PARTS, nq=4, ksb=KSB,
    trim=False,
):
    f32 = mybir.dt.float32
    f16 = mybir.dt.float16
    nb = epw // P
    big = cw * epw
    cwnb = cw * nb
    assert n_nodes % 2 == 0

    nc = bacc.Bacc(
        "TRN2", target_bir_lowering=False, debug=False, num_swdge_queues=nq
    )

    qp_t = nc.dram_tensor("qp", [NPAD, 2 * d], f16, kind="ExternalInput")
    qps_t = nc.dram_tensor("qps", [P, RANKS * 2 * d], f16, kind="ExternalInput")
    idx_t = nc.dram_tensor(
        "idx", [nchunk, 2, P, big // 16], mybir.dt.int16, kind="ExternalInput"
    )
    aux_t = nc.dram_tensor("aux", [nchunk, P, 4 * cwnb], f16, kind="ExternalInput")
    wgt_t = nc.dram_tensor("wgt", [nchunk, P, 2 * cwnb], f32, kind="ExternalInput")
    cnt_t = nc.dram_tensor(
        "cnt", [1, nchunk * 2 * cw], mybir.dt.int32, kind="ExternalInput"
    )
    iota_t = nc.dram_tensor("iota", [P, P], f16, kind="ExternalInput")
    ident_t = nc.dram_tensor("ident", [P, P], f16, kind="ExternalInput")
    out_t = nc.dram_tensor("out", [wpc * P, d], f32, kind="ExternalOutput")

    qview = qp_t.ap()[:, 0 : 2 * d]

    def bcast(ap2d, n_mid, mid_is_data):
        pairs = list(ap2d.ap)
        assert len(pairs) == 2
        if mid_is_data:
            newp = [pairs[0], [pairs[1][0], n_mid], [0, P]]
        else:
            newp = [pairs[0], [0, n_mid], pairs[1]]
        return bass.AP(ap2d.tensor, ap2d.offset, newp)

    def bcast64(ap2d, n_mid):
        pairs = list(ap2d.ap)
        assert len(pairs) == 2
        newp = [pairs[0], [pairs[1][0], n_mid], [0, 64]]
        return bass.AP(ap2d.tensor, ap2d.offset, newp)

    with tile.TileContext(nc) as tc:
        with (
            tc.tile_pool(name="const", bufs=1) as cpool,
            tc.tile_pool(name="io", bufs=2) as iopool,
            tc.tile_pool(name="gat", bufs=2) as gpool,
            tc.tile_pool(name="gts", bufs=2) as tpool,
            tc.tile_pool(name="gw", bufs=2) as wpool,
            tc.tile_pool(name="gwsb", bufs=10) as spool,
            tc.tile_pool(name="hot", bufs=3) as hpool,
            tc.tile_pool(name="ost", bufs=4) as opool,
            tc.tile_pool(name="ps", bufs=4, space="PSUM") as ppool,
            tc.tile_pool(name="pst", bufs=4, space="PSUM") as ppool2,
        ):
            iota_f = cpool.tile([P, P], f16)
            nc.sync.dma_start(out=iota_f[:], in_=iota_t.ap()[:, :])
            ident = cpool.tile([P, P], f16)
            nc.sync.dma_start(out=ident[:], in_=ident_t.ap()[:, :])
            qsb = cpool.tile([P, RANKS * 2 * d], f16)
            cnt = cpool.tile([1, nchunk * 2 * cw], mybir.dt.int32)
            nc.sync.dma_start(out=cnt[:], in_=cnt_t.ap()[:, :])
            # pre-zero HBM gather slots: reg-trimmed tails are never written
            # and must not hold NaN bit patterns on the first iteration
            for h in (0, 1):
                if cw > ksb[h]:
                    for _ in range(2):
                        gz = gpool.tile(
                            [P, (cw - ksb[h]) * nb, 2 * d], f16, tag=f"g{h}"
                        )
                        nc.vector.memset(gz[:], 0)

            loop = tc.For_i(0, iters, 1) if iters > 1 else contextlib.nullcontext()
            with loop:
                # stage the queue into SBUF (stripe-major node-pair tokens)
                if "gather" in parts:
                    nc.sync.dma_start(out=qsb[:], in_=qps_t.ap()[:, :])
                for c in range(nchunk):
                    idxs = []
                    for h in (0, 1):
                        it = iopool.tile(
                            [P, big // 16], mybir.dt.int16, tag=f"idx{h}"
                        )
                        nc.sync.dma_start(out=it[:], in_=idx_t.ap()[c, h])
                        idxs.append(it)
                    aux = iopool.tile([P, 4 * cwnb], f16, tag="aux")
                    nc.sync.dma_start(out=aux[:], in_=aux_t.ap()[c])
                    wf = None
                    if max(ksb) > 0:
                        wf = iopool.tile([P, 2 * cwnb], f32, tag="wgt")
                        nc.sync.dma_start(out=wf[:], in_=wgt_t.ap()[c])

                    sl = epw // 16
                    ghbm = []
                    gws = []
                    gsb = {}
                    # HBM-path gathers first: they are the long pole
                    for h in (0, 1):
                        k = ksb[h]
                        g = None
                        if cw > k:
                            g = gpool.tile(
                                [P, (cw - k) * nb, 2 * d], f16, tag=f"g{h}"
                            )
                        if "gather" in parts and cw > k:
                            nqh = 2 if max(ksb) > 0 else nq
                            for j in range(k, cw):
                                jj = j - k
                                if trim:
                                    r = nc.alloc_register(mybir.EngineType.Pool)
                                    gi = (c * 2 + h) * cw + j
                                    nc.gpsimd.reg_load(r, cnt[0:1, gi : gi + 1])
                                else:
                                    r = epw
                                nc.gpsimd.dma_gather(
                                    out_ap=g[:, jj * nb : (jj + 1) * nb, :],
                                    in_ap=qview,
                                    idxs_ap=idxs[h][:, j * sl : (j + 1) * sl],
                                    num_idxs=epw,
                                    num_idxs_reg=r,
                                    elem_size=2 * d,
                                    elem_step=2 * d,
                                    single_packet=False,
                                    queue_num=(j + h) % nqh,
                                )
                        ghbm.append(g)
                    # SBUF-path gathers: windows [0, k), window-level
                    for j in range(max(ksb)):
                        for h in (0, 1):
                            if j >= ksb[h]:
                                continue
                            gt = tpool.tile([P, 1, epw], f16, tag=f"gt{h}{j}")
                            if "gather" in parts:
                                nc.gpsimd.dma_gather(
                                    out_ap=gt[:],
                                    in_ap=qsb[:],
                                    idxs_ap=idxs[h][:, j * sl : (j + 1) * sl],
                                    num_idxs=epw,
                                    num_idxs_reg=epw,
                                    elem_size=2 * d,
                                    transpose=True,
                                    single_packet=False,
                                    queue_num=2 + ((j + h) % 2),
                                    sbuf_tokens_per_rank=P,
                                    sbuf_free_dim_per_rank=4 * d,
                                    sbuf_free_dim_pad_per_rank=0,
                                    sbuf_byte_offset=0,
                                )
                            gsb[(h, j)] = gt
                    for h in (0, 1):
                        k = ksb[h]
                        gw = None
                        if cw > k:
                            gw = wpool.tile([P, (cw - k) * nb, d], f16, tag=f"gw{h}")
                        if "dve" in parts and cw > k:
                            # gw = g[:, :, h*64:(h+1)*64] * w (per edge)
                            nc.vector.tensor_tensor(
                                out=gw[:],
                                in0=ghbm[h][:, :, h * d : (h + 1) * d],
                                in1=bcast64(
                                    aux[
                                        :,
                                        (2 + h) * cwnb + k * nb : (3 + h) * cwnb,
                                    ],
                                    (cw - k) * nb,
                                ),
                                op=mybir.AluOpType.mult,
                            )
                        gws.append(gw)

                    for j in range(cw):
                        nbw = 2 * nb
                        wcol = j * nbw
                        h01 = hpool.tile([P, nbw, P], f16, tag="h01")
                        if "dve" in parts:
                            nc.vector.tensor_tensor(
                                out=h01[:],
                                in0=bcast(iota_f[:], nbw, False),
                                in1=bcast(aux[:, wcol : wcol + nbw], nbw, True),
                                op=mybir.AluOpType.is_equal,
                            )
                        # SBUF-path rhs blocks: transpose + weight on ACT
                        rhs_sb = {}
                        for h in (0, 1):
                            if j >= ksb[h] or "gather" not in parts:
                                continue
                            gt = gsb[(h, j)]
                            for k2 in range(nb):
                                # transpose as a plain matmul: gt_blk.T @ I
                                # (full-bank PSUM tile: start=True clears the
                                # whole bank's has_written bits, so no other
                                # accumulator may share this bank)
                                pt = ppool2.tile([P, 512], f32, tag="pt")
                                nc.tensor.matmul(
                                    pt[:, 0:P],
                                    lhsT=gt[:, 0, k2 * P : (k2 + 1) * P],
                                    rhs=ident[:],
                                    start=True,
                                    stop=True,
                                )
                                gwb = spool.tile([P, 64], f16, tag=f"gwb{h}")
                                wc = h * cwnb + j * nb + k2
                                nc.scalar.mul(
                                    gwb[:],
                                    pt[:, h * 64 : (h + 1) * 64],
                                    wf[:, wc : wc + 1],
                                )
                                rhs_sb[(h, k2)] = gwb
                        ps = ppool.tile([P, 512], f32)
                        if "mm" in parts:
                            first = True
                            for h in (0, 1):
                                k = ksb[h]
                                for k2 in range(nb):
                                    bi = h * nb + k2
                                    lhs = (
                                        h01[:, bi, :]
                                        if "dve" in parts
                                        else iota_f[:]
                                    )
                                    if j < k and "gather" in parts:
                                        rhs = rhs_sb[(h, k2)][:]
                                    else:
                                        jj = max(j - k, 0)
                                        rhs = gws[h][:, jj * nb + k2, :]
                                    nc.tensor.matmul(
                                        ps[:, 0:d],
                                        lhsT=lhs,
                                        rhs=rhs,
                                        start=first,
                                        stop=(h == 1 and k2 == nb - 1),
                                    )
                                    first = False
                        wg = c * cw + j
                        if "out" in parts and "mm" in parts:
                            ot = opool.tile([P, d], f32, tag="ot")
                            nc.scalar.copy(ot[:], ps[:, 0:d])
                            nc.sync.dma_start(
                                out=out_t.ap()[wg * P : (wg + 1) * P, :], in_=ot[:]
                            )
                        elif "mm" not in parts:
                            if "dve" in parts:
                                srcs = [h01[:, 0, :]]
                            else:
                                srcs = [
                                    g[:, 0, 0:P] for g in ghbm if g is not None
                                ]
                            for src_ap in srcs:
                                nc.sync.dma_start(
                                    out=out_t.ap()[wg * P : (wg + 1) * P, 0:32],
                                    in_=src_ap.bitcast(f32)[:, 0:32],
                                )
    nc.compile()
    return nc


def _make_inputs(queue, idx_hbm, aux_hbm, wgt_hbm, cnt_hbm, n_cores):
    f16 = np.float16
    q = np.asarray(queue, dtype=np.float32)
    d = q.shape[1]
    qp = np.zeros((NPAD, 2 * d), f16)
    qp[:NPAIR, 0:d] = q[0::2].astype(f16)
    qp[:NPAIR, d : 2 * d] = q[1::2].astype(f16)
    # stripe-major staging image: token t -> partition t & 127,
    # free cols (t>>7)*2d .. +2d
    qps = np.ascontiguousarray(
        qp.reshape(RANKS, P, 2 * d).transpose(1, 0, 2).reshape(P, RANKS * 2 * d)
    )
    iota_np = np.ascontiguousarray(
        np.broadcast_to(np.arange(P, dtype=np.float32), (P, P)).astype(f16)
    )
    ident_np = np.ascontiguousarray(np.eye(P, dtype=f16))
    return [
        {
            "qp": qp,
            "qps": qps,
            "idx": idx_hbm[c],
            "aux": aux_hbm[c],
            "wgt": wgt_hbm[c],
            "cnt": cnt_hbm[c],
            "iota": iota_np,
            "ident": ident_np,
        }
        for c in range(n_cores)
    ]


def _run(queue, weight, src, dst, n_nodes, d, n_cores, iters=1):
    queue = np.ascontiguousarray(np.asarray(queue, dtype=np.float32))
    wpc, cw, nchunk = _plan(n_nodes, n_cores)
    epw, idx_hbm, aux_hbm, wgt_hbm, cnt_hbm = _host_prep(
        weight, src, dst, n_nodes, wpc, cw, nchunk, n_cores
    )
    nc = _build(n_nodes, d, epw, wpc, cw, nchunk, iters=iters)
    in_maps = _make_inputs(queue, idx_hbm, aux_hbm, wgt_hbm, cnt_hbm, n_cores)
    res = run_bass_kernel_spmd(nc, in_maps, core_ids=list(range(n_cores)))
    full = np.concatenate([res.results[c]["out"] for c in range(n_cores)], axis=0)
    return full[:n_nodes], res


def kernel(queue, weight, src, dst):
    out, _ = _run(queue, weight, src, dst, N_NODES, D_FEAT, N_CORES)
    return out
